# revision 4
# baseline (speedup 1.0000x reference)
"""Trainium2 Bass kernel for nn_CycleConsistencyLoss.

Math (per layer l with T = attn_t (B,H,NT,NV), V = attn_v (B,H,NV,NT)):
    P_t = T.mean(axis=1)                      # (B, NT, NV)
    P_v = V.mean(axis=1)                      # (B, NV, NT)
    td[b,i] = sum_k P_t[b,i,k] * P_v[b,k,i]   # (B, NT)
    vd[b,j] = sum_k P_v[b,j,k] * P_t[b,k,j]   # (B, NV)
    loss_l = 0.5*(mean(-log(clip(td))) + mean(-log(clip(vd))))
    loss   = mean over layers.

Sharding: data-parallel over batch, one batch element per NeuronCore
(B == 8 == n_cores). Each core returns 4 partial sums
[sum_i log(clip(td_l0)), sum_j log(clip(vd_l0)), same for l1]; the host
combines them into the scalar loss.

Per-core kernel structure (DMA-roofline bound: 75.5 MB of input/core):
  - interleaved row layout: partition p holds 4 consecutive matrix rows
    (4p..4p+3), giving 9216/8192-byte DMA descriptors (peak HBM rate);
  - head sums S_t = sum_h T[h], S_v = sum_h V[h]: streaming loads on two
    HWDGE queues (T on sync/SP, V on scalar/ACT) with tensor_add chains on
    the Pool engine (early heads) and DVE (late heads, shorter tail);
  - PE-array 128x128 block transposes (exact) produce S_v^T and S_t^T,
    strided APs keep the interleaved row order consistent;
  - fused DVE scalar_tensor_tensor computes (S_t * 1/H^2) * S_v^T with a
    per-partition row-sum accumulator -> td (and the V-orientation -> vd);
  - clip on DVE, Ln on ACT with free-axis accumulation, ones-matmul
    reduces across partitions to 4 scalars.
"""

import numpy as np

B, H, NT, NV = 8, 16, 512, 576
N_CORES = 8
EPS_MIN, EPS_MAX = 1e-8, 1.0
SCALE = 1.0 / (H * H)  # fold the two 1/H head-mean factors into the product
P = 128
R = 4                     # interleave: partition p holds rows 4p..4p+3
NV_REM = NV - P * R       # 64 remainder rows of V (512..575)
POOL_HEADS = 6            # heads [0,6) accumulate on Pool (early), rest DVE

_CACHE = {}


def _build_nc():
    import concourse.bacc as bacc
    import concourse.mybir as mybir
    import concourse.tile as tile

    f32 = mybir.dt.float32
    Alu = mybir.AluOpType
    Act = mybir.ActivationFunctionType

    nc = bacc.Bacc(
        "TRN2", target_bir_lowering=False, debug=False, num_devices=N_CORES
    )

    t0 = nc.dram_tensor("t0", [H, NT, NV], f32, kind="ExternalInput")
    v0 = nc.dram_tensor("v0", [H, NV, NT], f32, kind="ExternalInput")
    t1 = nc.dram_tensor("t1", [H, NT, NV], f32, kind="ExternalInput")
    v1 = nc.dram_tensor("v1", [H, NV, NT], f32, kind="ExternalInput")
    ident = nc.dram_tensor("ident", [P, P], f32, kind="ExternalInput")
    out_d = nc.dram_tensor("out", [4], f32, kind="ExternalOutput")

    from contextlib import ExitStack

    with tile.TileContext(nc) as tc, ExitStack() as ctx:
        pool = lambda name, bufs, **kw: ctx.enter_context(
            tc.tile_pool(name=name, bufs=bufs, **kw)
        )
        consts = pool("consts", 1)
        t_stream = pool("t_stream", 4)
        v_stream = pool("v_stream", 4)
        v64_stream = pool("v64_stream", 4)
        acc_t_d_p = pool("acc_t_d", 2)
        acc_t_p_p = pool("acc_t_p", 2)
        acc_v_d_p = pool("acc_v_d", 2)
        acc_v_p_p = pool("acc_v_p", 2)
        acc64_d_p = pool("acc64_d", 2)
        acc64_p_p = pool("acc64_p", 2)
        vt_p = pool("vt", 1)
        tt_p = pool("tt", 1)
        tt64_p = pool("tt64", 1)
        m_t_p = pool("m_t", 1)
        m_v_p = pool("m_v", 1)
        m64_p = pool("m64", 1)
        small = pool("small", 2)
        psum_tr = pool("psum_tr", 6, space="PSUM")
        psum_out_p = pool("psum_out", 1, space="PSUM")

        ident_sb = consts.tile([P, P], f32)
        nc.sync.dma_start(ident_sb[:], ident.ap())
        ones_sb = consts.tile([P, 1], f32)
        nc.vector.memset(ones_sb[:], 1.0)
        # per-partition Ln sums; cols = [td_l0, vd_l0, td_l1, vd_l1]
        ln_parts = consts.tile([P, 4], f32)

        for li, (t_dram, v_dram) in enumerate([(t0, v0), (t1, v1)]):
            # ---- head sums ----
            # S_t  [p, j*NV + k] = sum_h T[h, 4p+j, k]      (j < 4)
            # S_v  [q, m*NT + i] = sum_h V[h, 4q+m, i]      (m < 4)
            # S_v64 [r, i]       = sum_h V[h, 512+r, i]     (r < 64)
            acc_t_d = acc_t_d_p.tile([P, R * NV], f32)
            acc_t_p = acc_t_p_p.tile([P, R * NV], f32)
            acc_v_d = acc_v_d_p.tile([P, R * NT], f32)
            acc_v_p = acc_v_p_p.tile([P, R * NT], f32)
            acc64_d = acc64_d_p.tile([NV_REM, NT], f32)
            acc64_p = acc64_p_p.tile([NV_REM, NT], f32)

            def t_src(h):
                # row = 4p + j -> per-partition 4 consecutive rows (9216 B)
                return t_dram.ap()[h].rearrange("(p j) k -> p j k", j=R)

            def v_src(h):
                return v_dram.ap()[h, 0 : P * R, :].rearrange(
                    "(q m) i -> q m i", m=R
                )

            def v64_src(h):
                return v_dram.ap()[h, P * R : NV, :]

            def t_view(tile_):
                return tile_[:].rearrange("p (j k) -> p j k", k=NV)

            def v_view(tile_):
                return tile_[:].rearrange("q (m i) -> q m i", i=NT)

            for h in range(H):
                on_pool = h < POOL_HEADS
                eng = nc.gpsimd if on_pool else nc.vector
                if h == 0 or h == POOL_HEADS:
                    at = acc_t_p if on_pool else acc_t_d
                    av = acc_v_p if on_pool else acc_v_d
                    a64 = acc64_p if on_pool else acc64_d
                    nc.sync.dma_start(t_view(at), t_src(h))
                    nc.scalar.dma_start(v_view(av), v_src(h))
                    nc.scalar.dma_start(a64[:], v64_src(h))
                else:
                    at = acc_t_p if on_pool else acc_t_d
                    av = acc_v_p if on_pool else acc_v_d
                    a64 = acc64_p if on_pool else acc64_d
                    tl = t_stream.tile([P, R * NV], f32)
                    nc.sync.dma_start(t_view(tl), t_src(h))
                    eng.tensor_add(at[:], at[:], tl[:])
                    vl = v_stream.tile([P, R * NT], f32)
                    nc.scalar.dma_start(v_view(vl), v_src(h))
                    eng.tensor_add(av[:], av[:], vl[:])
                    v64l = v64_stream.tile([NV_REM, NT], f32)
                    nc.scalar.dma_start(v64l[:], v64_src(h))
                    eng.tensor_add(a64[:], a64[:], v64l[:])

            nc.vector.tensor_add(acc_t_d[:], acc_t_d[:], acc_t_p[:])
            nc.vector.tensor_add(acc_v_d[:], acc_v_d[:], acc_v_p[:])
            nc.vector.tensor_add(acc64_d[:], acc64_d[:], acc64_p[:])
            S_t, S_v, S_v64 = acc_t_d, acc_v_d, acc64_d

            # Strided sub-views for the transposes.
            # S_t j-slice over k<512 grouped by (q, m): k = 4q + m
            def st_j_km(j, m):
                return (
                    S_t[:, j * NV : j * NV + P * R]
                    .rearrange("p (q m) -> p m q", m=R)[:, m, :]
                )

            def st_j_k64(j):
                return S_t[:, j * NV + P * R : (j + 1) * NV]

            # S_v m-slice over i grouped by (c, j): i = 4c + j
            def sv_m_ij(m, j):
                return (
                    S_v[:, m * NT : (m + 1) * NT]
                    .rearrange("q (c j) -> q j c", j=R)[:, j, :]
                )

            def sv64_ij(j):
                return S_v64[:].rearrange("r (c j) -> r j c", j=R)[:, j, :]

            # ---- transposes on the PE array ----
            # VT[p, j*NV + k] = S_v_mat[k, 4p+j]
            VT = vt_p.tile([P, R * NV], f32)

            def vt_j_km(j, m):
                return (
                    VT[:, j * NV : j * NV + P * R]
                    .rearrange("p (q m) -> p m q", m=R)[:, m, :]
                )

            for j in range(R):
                for m in range(R):
                    tr = psum_tr.tile([P, P], f32)
                    # in: (q part; c free, i=4c+j) -> out[c, q] = S_v[4q+m, 4c+j]
                    nc.tensor.transpose(tr[:], sv_m_ij(m, j), ident_sb[:])
                    nc.scalar.copy(vt_j_km(j, m), tr[:])
                tr = psum_tr.tile([P, P], f32)
                # in: (r part; c free) -> out[c, r] = S_v64[r, 4c+j]
                nc.tensor.transpose(
                    tr[:, 0:NV_REM], sv64_ij(j), ident_sb[0:NV_REM, 0:NV_REM]
                )
                nc.scalar.copy(
                    VT[:, j * NV + P * R : (j + 1) * NV], tr[:, 0:NV_REM]
                )

            # TT[q, m*NT + i] = S_t_mat[i, 4q+m]; TT64[r, i] = S_t_mat[i, 512+r]
            TT = tt_p.tile([P, R * NT], f32)

            def tt_m_ij(m, j):
                return (
                    TT[:, m * NT : (m + 1) * NT]
                    .rearrange("q (c j) -> q j c", j=R)[:, j, :]
                )

            for m in range(R):
                for j in range(R):
                    tr = psum_tr.tile([P, P], f32)
                    # in: (p part; q free, k=4q+m) -> out[q, p] = S_t[4p+j, 4q+m]
                    nc.tensor.transpose(tr[:], st_j_km(j, m), ident_sb[:])
                    nc.scalar.copy(tt_m_ij(m, j), tr[:])
            TT64 = tt64_p.tile([NV_REM, NT], f32)

            def tt64_ij(j):
                return TT64[:].rearrange("r (c j) -> r j c", j=R)[:, j, :]

            for j in range(R):
                tr = psum_tr.tile([P, P], f32)
                # in: (p part; 64 free, k=512+r) -> out[r, p] = S_t[4p+j, 512+r]
                nc.tensor.transpose(tr[0:NV_REM, :], st_j_k64(j), ident_sb[:])
                nc.scalar.copy(tt64_ij(j), tr[0:NV_REM, :])

            # ---- products + diagonal sums ----
            td_all = small.tile([P, R], f32)
            vd_all = small.tile([P, R + 1], f32)
            nc.vector.memset(vd_all[NV_REM:P, R : R + 1], 1.0)
            for j in range(R):
                m = m_t_p.tile([P, NV], f32)
                nc.vector.scalar_tensor_tensor(
                    m[:],
                    S_t[:, j * NV : (j + 1) * NV],
                    SCALE,
                    VT[:, j * NV : (j + 1) * NV],
                    Alu.mult,
                    Alu.mult,
                    accum_out=td_all[:, j : j + 1],
                )
            for mi in range(R):
                m = m_v_p.tile([P, NT], f32)
                nc.vector.scalar_tensor_tensor(
                    m[:],
                    S_v[:, mi * NT : (mi + 1) * NT],
                    SCALE,
                    TT[:, mi * NT : (mi + 1) * NT],
                    Alu.mult,
                    Alu.mult,
                    accum_out=vd_all[:, mi : mi + 1],
                )
            m64 = m64_p.tile([NV_REM, NT], f32)
            nc.vector.scalar_tensor_tensor(
                m64[:],
                S_v64[:],
                SCALE,
                TT64[:],
                Alu.mult,
                Alu.mult,
                accum_out=vd_all[0:NV_REM, R : R + 1],
            )

            # ---- clip + Ln (+ free-axis accumulate) ----
            nc.vector.tensor_scalar(
                td_all[:], td_all[:], EPS_MIN, EPS_MAX, Alu.max, Alu.min
            )
            nc.vector.tensor_scalar(
                vd_all[:], vd_all[:], EPS_MIN, EPS_MAX, Alu.max, Alu.min
            )
            ln_t = small.tile([P, R], f32)
            ln_v = small.tile([P, R + 1], f32)
            nc.scalar.activation(
                ln_t[:],
                td_all[:],
                Act.Ln,
                accum_out=ln_parts[:, 2 * li : 2 * li + 1],
            )
            nc.scalar.activation(
                ln_v[:],
                vd_all[:],
                Act.Ln,
                accum_out=ln_parts[:, 2 * li + 1 : 2 * li + 2],
            )

        # ---- reduce across partitions: out[c] = sum_p ln_parts[p, c] ----
        ps = psum_out_p.tile([1, 4], f32)
        nc.tensor.matmul(ps[:], ones_sb[:], ln_parts[:], start=True, stop=True)
        out_sb = consts.tile([1, 4], f32)
        nc.scalar.copy(out_sb[:], ps[:])
        nc.sync.dma_start(out_d.ap(), out_sb[:])

    nc.compile()
    return nc


def _get_nc():
    if "nc" not in _CACHE:
        _CACHE["nc"] = _build_nc()
    return _CACHE["nc"]


def _in_maps(inputs):
    eye = np.eye(P, dtype=np.float32)
    maps = []
    for c in range(N_CORES):
        maps.append(
            {
                "t0": np.ascontiguousarray(inputs["attn_t_0"][c], dtype=np.float32),
                "v0": np.ascontiguousarray(inputs["attn_v_0"][c], dtype=np.float32),
                "t1": np.ascontiguousarray(inputs["attn_t_1"][c], dtype=np.float32),
                "v1": np.ascontiguousarray(inputs["attn_v_1"][c], dtype=np.float32),
                "ident": eye,
            }
        )
    return maps


def _combine(outs):
    # outs: (N_CORES, 4) = per-batch [sum ln td_l0, sum ln vd_l0, l1 ...]
    outs = np.asarray(outs, dtype=np.float64)
    td = outs[:, [0, 2]].sum(axis=0)  # per layer
    vd = outs[:, [1, 3]].sum(axis=0)
    loss_text = -td / (B * NT)
    loss_vision = -vd / (B * NV)
    loss = ((loss_text + loss_vision) * 0.5).sum() / 2.0
    return np.float32(loss)


def _run(inputs, trace=False, **kwargs):
    from concourse.bass_utils import run_bass_kernel_spmd

    nc = _get_nc()
    res = run_bass_kernel_spmd(
        nc, _in_maps(inputs), list(range(N_CORES)), trace=trace, **kwargs
    )
    outs = np.stack([r["out"] for r in res.results])
    return _combine(outs), res


def kernel(**inputs) -> np.ndarray:
    inputs = {k: np.asarray(v) for k, v in inputs.items()}
    loss, _ = _run(inputs, trace=False)
    return loss
